# revision 14
# baseline (speedup 1.0000x reference)
"""Expert-parallel MoE GLU FFN for Trainium2 (8 NeuronCores, Bass/Tile).

Strategy: one expert per core. The host routes each (token, slot) pair to
its expert's core, pads each expert batch to a common capacity C, and
pre-transposes x / weights so the device kernel needs no on-chip
transposes. Matmuls run in bf16 with fp32 PSUM accumulation; weights stay
resident in SBUF so HBM traffic is weights-once + streamed activations.

fp8 quarter-path: the first 2 (of 8) K-slices of the fc1 h-path run as ONE
fp8-e4m3 DoubleRow matmul per FF tile (K=256 per instruction at bf16
cycles/row = 2x flops), host-quantized as x8 = e4m3(4x), wh8 = e4m3(32*W1h).
The resulting x128 scale on h is folded host-side into the bf16 h columns
(x128) and into W2 (/128); the gate path is untouched, so sigmoid input is
exact. Measured end-to-end rel L2 vs the fp32 reference: 0.0191 (< 2e-2).

Per core, per token block (one 128-token lead block + 8x512):
  fc1 h:  DR fp8 (k=0,1) + 6 bf16 matmuls   (7 instr, N=block)
  fc1 g:  8 bf16 matmuls
  act:    a[ff,t] = (h * sigmoid(g)) * g     (ScalarE Sigmoid + 2 DVE muls)
  fc2:    o^T[dout,t] = w2t red @ a[ff,t]    (16 matmuls per dout block)
fc2 of block b is emitted after fc1 of block b+1 so the 4MB w2 DMA is off
the critical path of the small lead block. Output DMAs issue on Scalar/
Vector (GpSimd stays DMA-free: its end-of-kernel DMA drain costs ~3.8us).
"""

import sys

for _p in ("/opt/trn_rl_repo", "/opt/pypackages"):
    if _p not in sys.path:
        sys.path.append(_p)

import numpy as np
import ml_dtypes

import concourse.bass as bass
import concourse.mybir as mybir
import concourse.tile as tile
from concourse.bass_utils import run_bass_kernel_spmd

BF16 = ml_dtypes.bfloat16
E4M3 = ml_dtypes.float8_e4m3

N_CORES = 8
D_MODEL = 1024
D_FF = 2048
TBLK = 512          # tokens per block (one PSUM bank at fp32)
D_BLKS = D_MODEL // 128      # 8  contraction blocks for fc1
FF_BLKS = D_FF // 128        # 16 contraction blocks for fc2 / a tiles
DOUT_BLKS = D_MODEL // 128   # 8  output blocks
KFP8 = 2                     # leading fc1-h K-slices in fp8 DoubleRow
SX = 4.0                     # x fp8 scale
SW = 32.0                    # w1h fp8 scale
S = SX * SW                  # net h-path scale, folded into bf16 h / W2
# per-i bf16 w1 pack: k<KFP8 -> gate col only; k>=KFP8 -> [h|g]
W1COLS = KFP8 * 128 + (D_BLKS - KFP8) * 256


def _gcol(k):
    return 128 * k if k < KFP8 else KFP8 * 128 + (k - KFP8) * 256 + 128


def _hcol(k):
    assert k >= KFP8
    return KFP8 * 128 + (k - KFP8) * 256


def _fix_multiwaits(nc):
    """The walrus build in this env supports at most ONE sync-wait per
    instruction; split extras into single-wait NOPs placed just before the
    instruction on the same engine (same program order => same semantics)."""
    ctr = 0
    for f in nc.m.functions:
        for bb in f.blocks:
            out, changed = [], False
            for inst in bb.instructions:
                si = inst.sync_info
                waits = list(si.on_wait) if (si and si.on_wait) else []
                if len(waits) > 1:
                    changed = True
                    for w in waits[:-1]:
                        nop = mybir.InstNoOp(
                            name=f"mwfix_{ctr}",
                            engine=inst.engine,
                            sync_info=mybir.SyncInfo(on_wait=[w], on_update=[]),
                            bass_nofuse=True,
                        )
                        ctr += 1
                        nc.register_instruction(nop)
                        out.append(nop)
                    si.on_wait = [waits[-1]]
                out.append(inst)
            if changed:
                bb.instructions = out
    return ctr


def block_sizes(max_count):
    """128-token lead block (fast DMA start) + 512-token blocks + a
    128-granular tail block."""
    C = -(-max_count // 128) * 128
    if C <= 128:
        return [C]
    blocks = [128]
    rem = C - 128
    blocks += [TBLK] * (rem // TBLK)
    if rem % TBLK:
        blocks.append(rem % TBLK)
    return blocks


def _light_drain_and_barrier(self, tick_clock, wait_clock):
    """Tile epilogue minus the final all-engine barrier: the barrier after
    the sem clears only orders per-engine stream ends, which NEFF completion
    already requires, and the preamble of any later execution re-clears and
    barriers before the body runs. Saves ~3us of EVSEM butterfly."""
    import bass_rust

    nc = self.nc
    drain_inst = nc.sync.drain()
    wait_clock.add_sem_waits(
        drain_inst.ins, bass_rust.ScopedClock({None: tick_clock.global_clock})
    )
    nc.all_engine_barrier()
    popped = nc._tile_sem_poison_stack.pop()
    assert popped is self._sem_poison
    # bookkeeping only — skip clear_and_free_semaphores' dma_reset (a ~3.8us
    # gpsimd DRAIN) and range clear: the kernel postamble zeroes every
    # semaphore right after this anyway.
    sems = list(self.sems.allocated().values())
    sem_nums = [s.num if hasattr(s, "num") else s for s in sems]
    nc._state.prepend_free_semaphores(sem_nums)
    for poison_set in nc._tile_sem_poison_stack:
        poison_set.update(sem_nums)


def build_bass(blocks):
    """Build the per-core Bass program for the given token-block sizes.

    Packed DRAM layouts (DMA issue costs ~0.6us of sequencer time each, so
    few large 2D DMAs):
      w1c: [FF_BLKS, 128, W1COLS] bf16; cols = k-major: k<KFP8 gate-only
           (128), k>=KFP8 [h(x S)|gate] (256)
      wh8: [128, 2, FF_BLKS*128] fp8; [p, j, i*128+m] = e4m3(SW*W1h[m,d])
      w2c: [128, FF_BLKS*D_MODEL] bf16 (/S); cols = i-major packs of dout
      xc:  [128, D_BLKS*C] bf16 block-major, k-major inside
      x8c: [128, 2*C] fp8 block-major; per block [j=0 cols | j=1 cols]
    """
    C = sum(blocks)
    f32 = mybir.dt.float32
    bf16 = mybir.dt.bfloat16
    fp8 = mybir.dt.float8e4
    DR = mybir.MatmulPerfMode.DoubleRow

    tile.TileContext._drain_and_barrier = _light_drain_and_barrier

    nc = bass.Bass()
    xc = nc.declare_dram_parameter("xc", [128, D_BLKS * C], bf16, isOutput=False)
    x8c = nc.declare_dram_parameter("x8c", [128, 2 * C], fp8, isOutput=False)
    w1c = nc.declare_dram_parameter(
        "w1c", [FF_BLKS, 128, W1COLS], bf16, isOutput=False
    )
    wh8c = nc.declare_dram_parameter(
        "wh8", [128, 2, FF_BLKS * 128], fp8, isOutput=False
    )
    w2c = nc.declare_dram_parameter(
        "w2c", [128, FF_BLKS * D_MODEL], bf16, isOutput=False
    )
    yt = nc.declare_dram_parameter("yt", [D_MODEL, C], f32, isOutput=True)

    nb = len(blocks)
    starts = [sum(blocks[:b]) for b in range(nb)]

    with tile.TileContext(nc) as tc:
        with (
            tc.tile_pool(name="weights", bufs=1) as wpool,
            tc.tile_pool(name="xin", bufs=2) as xpool,
            tc.tile_pool(name="act", bufs=2) as apool,
            tc.tile_pool(name="out", bufs=4) as opool,
            tc.tile_pool(name="psum", bufs=3, space="PSUM") as psum,
            tc.tile_pool(name="psum_o", bufs=2, space="PSUM") as psum_o,
        ):
            # HAM warm-up: dependency-free dummy matmuls fill the dead head
            # (waiting for the first x/weight DMAs) with PE activity so the
            # clock is ramped when the first real matmul issues.
            warm = apool.tile([128, 128], bf16, tag="warm")
            nc.vector.memset(warm[:], 0.0)
            pwarm = psum_o.tile([128, 128], f32, tag="po")
            for _ in range(22):
                nc.tensor.matmul(pwarm[:], warm[:], warm[:], start=True, stop=True)

            def load_x(ts, tn):
                t = xpool.tile([128, D_BLKS * tn], bf16, tag="x")
                nc.sync.dma_start(t[:], xc[:, D_BLKS * ts:D_BLKS * (ts + tn)])
                return t

            def load_x8(ts, tn):
                t = xpool.tile([128, 2, tn], fp8, tag="x8")
                nc.sync.dma_start(t[:], x8c[:, 2 * ts:2 * (ts + tn)])
                return t

            # critical-path first: block-0 fp8 x, fp8 h-weights, block-0
            # bf16 x, w1c[0] (split so its k<KFP8 gate cols land first),
            # then block-1 x AHEAD of the remaining ~11MB of weights (the
            # small block 0 finishes fc1 in ~25us; x_1 must not queue
            # behind w1_1..15 + w2)
            x8_first = load_x8(0, blocks[0])
            wh8_sb = wpool.tile([128, 2, FF_BLKS * 128], fp8, tag="wh8")
            nc.sync.dma_start(wh8_sb[:, :, 0:4 * 128], wh8c[:, :, 0:4 * 128])
            x_first = load_x(0, blocks[0])

            w1_sb = []
            xq = {}
            for i in range(FF_BLKS):
                t = wpool.tile([128, W1COLS], bf16, tag=f"w1_{i}")
                if i == 0:
                    nc.sync.dma_start(
                        t[:, 0:KFP8 * 128], w1c[i][:, 0:KFP8 * 128]
                    )
                    nc.sync.dma_start(
                        t[:, KFP8 * 128:], w1c[i][:, KFP8 * 128:]
                    )
                else:
                    nc.sync.dma_start(t[:], w1c[i])
                w1_sb.append(t)
                if i == 4:
                    # block 0 consumes w1_1..4 first; x_1 isn't needed
                    # until block 0's fc1 completes (~34us)
                    nc.sync.dma_start(
                        wh8_sb[:, :, 4 * 128:], wh8c[:, :, 4 * 128:]
                    )
                    if nb > 1:
                        xq[1] = (
                            load_x(starts[1], blocks[1]),
                            load_x8(starts[1], blocks[1]),
                        )
            w2_sb = wpool.tile([128, FF_BLKS * D_MODEL], bf16, tag="w2")
            nc.sync.dma_start(w2_sb[:], w2c[:])

            a_tiles = [None] * nb

            def fc1(tb):
                ts, tn = starts[tb], blocks[tb]
                if tb == 0:
                    x_sb, x8_sb = x_first, x8_first
                else:
                    x_sb, x8_sb = xq.pop(tb)
                a_sb = []
                for i in range(FF_BLKS):
                    ph = psum.tile([128, tn], f32, tag="ph")
                    nc.tensor.matmul(
                        ph[:], wh8_sb[:, :, i * 128:(i + 1) * 128], x8_sb[:],
                        start=True, stop=False, perf_mode=DR,
                    )
                    for k in range(KFP8, D_BLKS):
                        hc = _hcol(k)
                        nc.tensor.matmul(
                            ph[:], w1_sb[i][:, hc:hc + 128],
                            x_sb[:, k * tn:(k + 1) * tn],
                            start=False, stop=(k == D_BLKS - 1),
                        )
                    pg = psum.tile([128, tn], f32, tag="pg")
                    for k in range(D_BLKS):
                        gc = _gcol(k)
                        nc.tensor.matmul(
                            pg[:], w1_sb[i][:, gc:gc + 128],
                            x_sb[:, k * tn:(k + 1) * tn],
                            start=(k == 0), stop=(k == D_BLKS - 1),
                        )
                    # a = (S*h) * silu(g) = ((S*h) * sigmoid(g)) * g, each
                    # DVE op reading a single PSUM operand
                    sg = apool.tile([128, tn], f32, tag="sg", bufs=3)
                    nc.scalar.activation(
                        sg[:], pg[:], mybir.ActivationFunctionType.Sigmoid
                    )
                    hs = apool.tile([128, tn], f32, tag="hs", bufs=3)
                    nc.vector.tensor_mul(hs[:], ph[:], sg[:])
                    a = apool.tile([128, tn], bf16, tag=f"a_{i}")
                    nc.vector.tensor_mul(a[:], pg[:], hs[:])
                    a_sb.append(a)
                a_tiles[tb] = a_sb
                # prefetch next block's x HERE: later (in fc2) the sync
                # sequencer stalls on output-DMA sem waits, which would
                # delay the x issue past its use
                if tb + 1 < nb and tb + 1 not in xq:
                    nts, ntn = starts[tb + 1], blocks[tb + 1]
                    xq[tb + 1] = (load_x(nts, ntn), load_x8(nts, ntn))

            def fc2(tb):
                ts, tn = starts[tb], blocks[tb]
                a_sb = a_tiles[tb]
                for j in range(DOUT_BLKS):
                    po = psum_o.tile([128, tn], f32, tag="po")
                    for i in range(FF_BLKS):
                        nc.tensor.matmul(
                            po[:],
                            w2_sb[:, i * D_MODEL + j * 128:i * D_MODEL + (j + 1) * 128],
                            a_sb[i][:],
                            start=(i == 0), stop=(i == FF_BLKS - 1),
                        )
                    o = opool.tile([128, tn], f32, tag="o")
                    nc.scalar.copy(o[:], po[:])
                    # GpSimd must stay DMA-free (its epilogue DMA drain costs
                    # ~3.8us); alternate Scalar/Sync for issue overlap
                    eng = nc.scalar if j % 2 == 0 else nc.sync
                    eng.dma_start(yt[j * 128:(j + 1) * 128, ts:ts + tn], o[:])
                a_tiles[tb] = None

            # fc2(b) emitted after fc1(b+1): block 0 is small, so its fc2
            # would otherwise stall on the w2 DMA (issued after all of w1)
            fc1(0)
            for tb in range(1, nb):
                fc1(tb)
                fc2(tb - 1)
            fc2(nb - 1)

    _fix_multiwaits(nc)
    return nc


# test harness hooks: test.py sets _RUN_KWARGS = {"trace": True, ...} to
# profile; LAST_RESULT then carries exec_time_ns / trace paths.
_RUN_KWARGS = {}
LAST_RESULT = None

# blocks-tuple -> (runner, out_name); reuses the compiled NEFF across
# kernel() calls so only the first call pays the neuronxcc compile.
_EXEC_CACHE = {}


def _get_runner(blocks):
    key = tuple(blocks)
    if key in _EXEC_CACHE:
        return _EXEC_CACHE[key]
    import jax
    from jax.experimental.shard_map import shard_map
    from jax.sharding import Mesh, PartitionSpec
    from concourse import bass2jax

    nc = build_bass(list(blocks))
    bass2jax.install_neuronx_cc_hook()

    partition_name = (
        nc.partition_id_tensor.name if nc.partition_id_tensor else None
    )
    in_names, out_names, out_avals, zero_shapes = [], [], [], []
    for alloc in nc.m.functions[0].allocations:
        if not isinstance(alloc, mybir.MemoryLocationSet):
            continue
        name = alloc.memorylocations[0].name
        if alloc.kind == "ExternalInput":
            if name != partition_name:
                in_names.append(name)
        elif alloc.kind == "ExternalOutput":
            out_names.append(name)
            shape = tuple(alloc.tensor_shape)
            dtype = mybir.dt.np(alloc.dtype)
            out_avals.append(jax.core.ShapedArray(shape, dtype))
            zero_shapes.append((shape, dtype))
    n_params = len(in_names)
    n_outs = len(out_names)
    all_names = in_names + out_names
    if partition_name is not None:
        all_names = all_names + [partition_name]
    donate = tuple(range(n_params, n_params + n_outs))

    def _body(*args):
        operands = list(args)
        if partition_name is not None:
            operands.append(bass2jax.partition_id_tensor())
        outs = bass2jax._bass_exec_p.bind(
            *operands,
            out_avals=tuple(out_avals),
            in_names=tuple(all_names),
            out_names=tuple(out_names),
            lowering_input_output_aliases=(),
            sim_require_finite=True,
            sim_require_nnan=True,
            nc=nc,
        )
        return tuple(outs)

    devices = jax.devices()[:N_CORES]
    mesh = Mesh(np.asarray(devices), ("core",))
    sharded = jax.jit(
        shard_map(
            _body,
            mesh=mesh,
            in_specs=(PartitionSpec("core"),) * (n_params + n_outs),
            out_specs=(PartitionSpec("core"),) * n_outs,
            check_rep=False,
        ),
        donate_argnums=donate,
        keep_unused=True,
    )

    def runner(in_maps):
        concat_in = [
            np.concatenate([np.asarray(m[name]) for m in in_maps], axis=0)
            for name in in_names
        ]
        concat_zeros = [
            np.zeros((N_CORES * s[0], *s[1:]), dt) for s, dt in zero_shapes
        ]
        out_arrs = sharded(*concat_in, *concat_zeros)
        return [
            {
                name: np.asarray(out_arrs[i]).reshape(
                    N_CORES, *out_avals[i].shape
                )[c]
                for i, name in enumerate(out_names)
            }
            for c in range(N_CORES)
        ]

    _EXEC_CACHE[key] = runner
    return runner


def _route(indices):
    """Group (token, slot) pairs by expert. Returns (order, starts, counts):
    order = pair indices sorted by expert (stable), starts = prefix offsets."""
    flat = np.asarray(indices).reshape(-1).astype(np.int64)
    order = np.argsort(flat, kind="stable")
    counts = np.bincount(flat, minlength=N_CORES)
    starts = np.zeros(N_CORES + 1, dtype=np.int64)
    np.cumsum(counts, out=starts[1:])
    return order, starts, counts


def kernel(x, fc1_weight, fc2_weight, indices, counts):
    x = np.asarray(x)
    fc1_weight = np.asarray(fc1_weight)
    fc2_weight = np.asarray(fc2_weight)
    n_tok, d_model = x.shape
    assert d_model == D_MODEL

    order, starts, cnt = _route(indices)
    top_k = np.asarray(indices).shape[-1]
    blocks = block_sizes(max(128, int(cnt.max())))
    C = sum(blocks)

    xb = x.astype(BF16)
    x8 = (x * SX).astype(E4M3)
    tok_of_pair = order // top_k

    in_maps = []
    for e in range(N_CORES):
        rows = tok_of_pair[starts[e]:starts[e + 1]]
        xe = np.zeros((C, D_MODEL), dtype=BF16)
        xe[: len(rows)] = xb[rows]
        # xc[p, 8*ts + k*tn + t] = xe[ts+t, k*128+p], per-block k-major
        xct = xe.T.reshape(D_BLKS, 128, C)          # (k, p, t)
        xc = np.empty((128, D_BLKS * C), dtype=BF16)
        # x8c[p, 2*ts + j*tn + t] = e4m3(SX * x[ts+t, j*128+p])
        x8e = np.zeros((C, 2 * 128), dtype=E4M3)
        x8e[: len(rows)] = x8[rows, : 2 * 128]
        x8t = x8e.T.reshape(2, 128, C)              # (j, p, t)
        x8c = np.empty((128, 2 * C), dtype=E4M3)
        ts = 0
        for tn in blocks:
            blk = xct[:, :, ts:ts + tn]             # (k, p, tn)
            xc[:, D_BLKS * ts:D_BLKS * (ts + tn)] = (
                blk.transpose(1, 0, 2).reshape(128, D_BLKS * tn)
            )
            b8 = x8t[:, :, ts:ts + tn]              # (j, p, tn)
            x8c[:, 2 * ts:2 * (ts + tn)] = (
                b8.transpose(1, 0, 2).reshape(128, 2 * tn)
            )
            ts += tn
        w1t = fc1_weight[e].T.astype(np.float32)    # (d, f) fp32
        h = w1t[:, :D_FF].reshape(D_BLKS, 128, FF_BLKS, 128)
        g = w1t[:, D_FF:].reshape(D_BLKS, 128, FF_BLKS, 128)
        # bf16 pack per i: [g(k=0) | g(1) | h*S|g (k=2) | ... | h*S|g (k=7)]
        w1c = np.empty((FF_BLKS, 128, W1COLS), dtype=BF16)
        for k in range(D_BLKS):
            gi = g[k].transpose(1, 0, 2)            # (i, p, m)
            gc = _gcol(k)
            w1c[:, :, gc:gc + 128] = gi.astype(BF16)
            if k >= KFP8:
                hc = _hcol(k)
                hi = (h[k] * S).transpose(1, 0, 2)
                w1c[:, :, hc:hc + 128] = hi.astype(BF16)
        # wh8[p, j, i*128+m] = e4m3(SW * h[k=j, p, i, m])
        wh8 = np.ascontiguousarray(
            (h[:KFP8] * SW).transpose(1, 0, 2, 3).reshape(128, 2, FF_BLKS * 128)
        ).astype(E4M3)
        # w2c[p, i*D_MODEL + dout] = W2[dout, i*128+p] / S
        w2t = (fc2_weight[e].T.astype(np.float32) / S).astype(BF16)
        w2c = np.ascontiguousarray(
            w2t.reshape(FF_BLKS, 128, D_MODEL)
            .transpose(1, 0, 2)
            .reshape(128, FF_BLKS * D_MODEL)
        )
        in_maps.append(
            {"xc": xc, "x8c": x8c, "w1c": w1c, "wh8": wh8, "w2c": w2c}
        )

    if _RUN_KWARGS:
        # profiling path (test harness only)
        nc = build_bass(blocks)
        res = run_bass_kernel_spmd(nc, in_maps, list(range(N_CORES)), **_RUN_KWARGS)
        global LAST_RESULT
        LAST_RESULT = res
        results = res.results
    else:
        results = _get_runner(tuple(blocks))(in_maps)

    out = np.zeros((n_tok * top_k, d_model), dtype=np.float32)
    for e in range(N_CORES):
        n_e = int(cnt[e])
        if n_e == 0:
            continue
        yt = np.asarray(results[e]["yt"])  # (D_MODEL, C) f32
        out[order[starts[e]:starts[e + 1]]] = yt.T[:n_e]
    return out
